# revision 4
# baseline (speedup 1.0000x reference)
"""NT-Xent (SimCLR) contrastive loss on 8 Trainium2 NeuronCores.

Strategy (SPMD + fp8 AllGather):
  z = normalize(concat(emb_i, emb_j))  # [8192, 512]
  Core c owns the 1024-row block starting at row 1024*c. The host hands each
  core ONLY its block (2 MiB); the core normalizes + quantizes it to fp8
  (scale 16), stages 512 KiB to DRAM, and an AllGather shares all blocks
  (4 MiB). Every core then works in a "rotated" local frame -- local rows
  0..1023 are its own block -- by reading the gathered buffer at
  partition-id-dependent octant offsets during the transposes. The positive
  pair of local row i is local row (i + 4096) % 8192 on every core, so the
  compute program is identical across cores (static matmul/ldweights APs).

  Per core (fp8 pipeline):
    - load own emb rows as row-PAIRS per partition ([pair, parity, d]);
      fp32 norms via fused DVE square+reduce (pre-scaled by 1/256 so
      exp(-0.5*ln(.)) yields 16/||e||)
    - DVE quantize z*16 -> fp8e4, parities byte-interleaved so the staged
      DRAM image S[pair, 2d+parity] is contiguous
    - AllGather fp8 blocks -> S_all [8, 512, 1024] (Shared DRAM)
    - u16-view DMA-xbar transposes (dynamic source octant = (local+pid)%8)
      build plane-separated fp8 zT [128, 4(k-plane), 8192(rows)] in SBUF
    - sim row-block via fp8 DoubleRow matmuls (256-deep contraction per
      instruction): psum = 256*sim in [128, 4x512] 4-bank PSUM tiles
    - one ACT exp(psum * 2/256) over 2048 elems with free-dim accumulation
      per tile -> row denominators (exp matrix never stored)
    - self-dot and positive-pair dot extracted from the PSUM diagonal via
      DVE identity-mask multiply+reduce
    - loss_row = ln(denom - exp(2*selfdot)) - 2*posdot
  Host: gather 8x1024 row losses, mean.
"""

import numpy as np

import concourse.bacc as bacc
import concourse.tile as tile
from concourse import mybir
from concourse.bass import ts
from concourse.bass_utils import run_bass_kernel_spmd

N_CORES = 8
D = 512
ROWS = 8192
BLK = ROWS // N_CORES  # 1024
P = 128
BLK_CHUNKS = BLK // P  # 8
NT = 512  # one PSUM bank of fp32
KD = D // P  # 4 contraction planes
SCALE = 16.0  # fp8 quantization scale; psum = SCALE^2 * sim
PSUM_SCALE = SCALE * SCALE

f32 = mybir.dt.float32
bf16 = mybir.dt.bfloat16
fp8 = mybir.dt.float8e4
u16 = mybir.dt.uint16
i32 = mybir.dt.int32

_ACT_PATCHED = False


def _patch_act_tables():
    """Make Exp and Ln resolve only to natural_log_exp_and_others so the
    whole kernel uses a single activation-table set."""
    global _ACT_PATCHED
    if _ACT_PATCHED:
        return
    import concourse.hw_specs as hw_specs

    Act = mybir.ActivationFunctionType
    orig = hw_specs.get_activation_tables("gen3")
    patched = {}
    for name, funcs in orig.items():
        fs = set(funcs)
        if name != "natural_log_exp_and_others":
            fs.discard(Act.Exp)
            fs.discard(Act.Ln)
        patched[name] = fs
    bacc.get_activation_tables = lambda arch: patched
    _ACT_PATCHED = True


def _build(loop_k: int = 1):
    _patch_act_tables()
    nc = bacc.Bacc("TRN2", target_bir_lowering=False, num_devices=N_CORES)
    emb = nc.dram_tensor("emb", [BLK, D], f32, kind="ExternalInput")
    loss = nc.dram_tensor("loss", [P, BLK_CHUNKS], f32, kind="ExternalOutput")
    S_own = nc.dram_tensor("S_own", [BLK // 2, 2 * D], fp8, kind="Internal")
    S_all = nc.dram_tensor(
        "S_all", [N_CORES, BLK // 2, 2 * D], fp8, kind="Internal", addr_space="Shared"
    )

    with tile.TileContext(nc) as tc:
        with (
            tc.tile_pool(name="persist", bufs=1) as persist,
            tc.tile_pool(name="loads", bufs=2) as loads,
            tc.tile_pool(name="zqs", bufs=2) as zqs,
            tc.tile_pool(name="scratch", bufs=3) as scratch,
            tc.tile_pool(name="small", bufs=2) as small,
            tc.tile_pool(name="psum", bufs=2, space="PSUM") as psum_pool,
        ):
            import contextlib

            loop_ctx = (
                tc.For_i(0, loop_k, 1) if loop_k > 1 else contextlib.nullcontext()
            )
            with loop_ctx:
                _body(
                    nc, tc, persist, loads, zqs, scratch, small, psum_pool,
                    emb, loss, S_own, S_all,
                )

    nc.compile()
    return nc


def _body(nc, tc, persist, loads, zqs, scratch, small, psum_pool, emb, loss, S_own, S_all):
    Alu = mybir.AluOpType
    Act = mybir.ActivationFunctionType

    # persistent tensors
    zT16 = persist.tile([P, KD, ROWS // 2], u16, tag="zT16")  # 32 KiB/part
    acc = [
        persist.tile([P, 4], f32, tag=f"acc{m}", name=f"acc{m}")
        for m in range(BLK_CHUNKS)
    ]
    selfd = persist.tile([P, BLK_CHUNKS], f32, tag="selfd")
    posd = persist.tile([P, BLK_CHUNKS], f32, tag="posd")
    ident = persist.tile([P, P], bf16, tag="ident")

    # identity mask for PSUM diagonal extraction: 1.0 at [p, p]
    io = small.tile([P, P], i32, tag="io")
    nc.gpsimd.iota(io, pattern=[[1, P]], base=0, channel_multiplier=-1)
    nc.vector.tensor_scalar(
        out=ident, in0=io, scalar1=0, scalar2=None, op0=Alu.is_equal
    )

    # ---- prologue: normalize + quantize OWN block, stage, AllGather ----
    sq = small.tile([P, 8], f32, tag="sq")
    ets = []
    for half in range(2):
        et = loads.tile([P, 2, 2, D], f32, tag="et")
        r0 = half * 512
        src = emb[r0 : r0 + 512, :].rearrange(
            "(cg p two) d -> p cg two d", p=P, two=2
        )
        nc.sync.dma_start(out=et, in_=src)
        ets.append(et)
    for half in range(2):
        for cg in range(2):
            for par in range(2):
                c = half * 4 + cg * 2 + par
                tt = scratch.tile([P, D], bf16, tag="ttout")
                nc.vector.scalar_tensor_tensor(
                    out=tt,
                    in0=ets[half][:, cg, par, :],
                    scalar=1.0 / PSUM_SCALE,
                    in1=ets[half][:, cg, par, :],
                    op0=Alu.mult,
                    op1=Alu.mult,
                    accum_out=sq[:, c : c + 1],
                )
    # SCALE/sqrt(s) = exp(-0.5*ln(s/SCALE^2)) -- single ACT table set
    lnv = small.tile([P, 8], f32, tag="lnv")
    nc.scalar.activation(out=lnv, in_=sq, func=Act.Ln)
    rinv = small.tile([P, 8], f32, tag="rinv")
    nc.scalar.activation(out=rinv, in_=lnv, func=Act.Exp, scale=-0.5)

    zq = zqs.tile([P, 4, 2 * D], fp8, tag="zq")
    for half in range(2):
        for cg in range(2):
            g = half * 2 + cg
            zq_pairs = zq[:, g, :].rearrange("p (d two) -> p two d", two=2)
            for par in range(2):
                c = half * 4 + cg * 2 + par
                nc.vector.tensor_scalar_mul(
                    out=zq_pairs[:, par, :],
                    in0=ets[half][:, cg, par, :],
                    scalar1=rinv[:, c : c + 1],
                )
    # stage own block (512 KiB, contiguous)
    nc.scalar.dma_start(
        out=S_own[:, :].rearrange("(g p) b -> p g b", p=P), in_=zq
    )
    # share fp8 blocks across all cores (4 MiB gathered)
    nc.gpsimd.collective_compute(
        "AllGather",
        Alu.bypass,
        replica_groups=[list(range(N_CORES))],
        ins=[S_own[:, :]],
        outs=[S_all[:, :, :]],
    )

    # ---- rotated transposes: local octant o reads global octant (o+pid)%8
    pid = nc.sync.partition_id()
    S16 = S_all[:, :, :].bitcast(u16).rearrange("g r q -> (g r) q")  # [4096, 512]
    octant_order = [0, 4, 1, 2, 3, 5, 6, 7]
    for oct_ in octant_order:
        g_dyn = (pid + oct_) % N_CORES
        for t in range(KD):
            nc.sync.dma_start_transpose(
                out=zT16[:, t, oct_ * 512 : (oct_ + 1) * 512],
                in_=S16[ts(g_dyn, 512), t * P : (t + 1) * P],
            )

    # ---- main loop: fp8 DoubleRow matmuls + fused exp/accumulate ----
    zT8 = zT16[:, :, :].bitcast(fp8)  # [128, KD, 8192]
    n_groups = [[0, 1, 8, 9], [2, 3, 4, 5], [6, 7, 10, 11], [12, 13, 14, 15]]
    for ng, group in enumerate(n_groups):
        gw = len(group)
        for m in range(BLK_CHUNKS):
            pst = psum_pool.tile([P, gw, NT], f32, tag="ps", bufs=2)
            for h in range(KD // 2):
                for li, n in enumerate(group):
                    nc.tensor.matmul(
                        pst[:, li, :],
                        zT8[:, 2 * h : 2 * h + 2, m * P : (m + 1) * P],
                        zT8[:, 2 * h : 2 * h + 2, n * NT : (n + 1) * NT],
                        start=(h == 0),
                        stop=(h == KD // 2 - 1),
                        perf_mode=mybir.MatmulPerfMode.DoubleRow,
                    )
            if ng == 0:
                # self (n-tiles 0,1) and positive-pair (n-tiles 8,9) dots
                # for rows m*128+p, straight from the PSUM diagonal.
                off = 128 * (m % 4)
                for li, dst_t in ((m // 4, selfd), (2 + m // 4, posd)):
                    dd = scratch.tile([P, P], bf16, tag="ddum")
                    nc.vector.scalar_tensor_tensor(
                        out=dd,
                        in0=pst[:, li, off : off + P],
                        scalar=1.0,
                        in1=ident,
                        op0=Alu.mult,
                        op1=Alu.mult,
                        accum_out=dst_t[:, m : m + 1],
                    )
            ex = scratch.tile([P, gw, NT], bf16, tag="exout")
            nc.scalar.activation(
                out=ex,
                in_=pst,
                func=Act.Exp,
                scale=2.0 / PSUM_SCALE,
                accum_out=acc[m][:, ng : ng + 1],
            )

    # ---- finale: loss_row = ln(denom - exp(2*selfdot)) - 2*posdot ----
    dsum = persist.tile([P, BLK_CHUNKS], f32, tag="dsum")
    for m in range(BLK_CHUNKS):
        nc.vector.reduce_sum(
            out=dsum[:, m : m + 1], in_=acc[m], axis=mybir.AxisListType.X
        )
    sexp = small.tile([P, BLK_CHUNKS], f32, tag="sexp")
    nc.scalar.activation(out=sexp, in_=selfd, func=Act.Exp, scale=2.0 / PSUM_SCALE)
    dx = small.tile([P, BLK_CHUNKS], f32, tag="dx")
    nc.vector.tensor_sub(dx, dsum, sexp)
    ld = small.tile([P, BLK_CHUNKS], f32, tag="ld")
    nc.scalar.activation(out=ld, in_=dx, func=Act.Ln)
    lossv = small.tile([P, BLK_CHUNKS], f32, tag="lossv")
    nc.vector.scalar_tensor_tensor(
        out=lossv,
        in0=posd,
        scalar=-2.0 / PSUM_SCALE,
        in1=ld,
        op0=Alu.mult,
        op1=Alu.add,
    )
    nc.sync.dma_start(out=loss[:, :], in_=lossv)


_NC_CACHE = []


def _get_nc():
    if not _NC_CACHE:
        _NC_CACHE.append(_build())
    return _NC_CACHE[0]


def make_in_maps(emb_i: np.ndarray, emb_j: np.ndarray):
    emb_all = np.concatenate(
        [np.asarray(emb_i, np.float32), np.asarray(emb_j, np.float32)], axis=0
    )
    return [
        {"emb": np.ascontiguousarray(emb_all[c * BLK : (c + 1) * BLK])}
        for c in range(N_CORES)
    ]


def assemble(results) -> np.ndarray:
    rows = []
    for c in range(N_CORES):
        out = results[c]["loss"]  # [128, 8]; out[p, m] = loss of block row m*128+p
        rows.append(out.T.reshape(-1))
    all_rows = np.concatenate(rows)  # original row order
    return np.float32(all_rows.astype(np.float64).mean())


def kernel(emb_i: np.ndarray, emb_j: np.ndarray) -> np.ndarray:
    nc = _get_nc()
    res = run_bass_kernel_spmd(
        nc, make_in_maps(emb_i, emb_j), core_ids=list(range(N_CORES))
    )
    return assemble(res.results)


if __name__ == "__main__":
    rng = np.random.default_rng(0)
    ei = rng.standard_normal((4096, D)).astype(np.float32)
    ej = rng.standard_normal((4096, D)).astype(np.float32)
    print(kernel(ei, ej))
